# revision 12
# baseline (speedup 1.0000x reference)
"""Trainium2 Bass kernel for nn_FCGFAutoencoder (segment_max -> 3-layer MLP decoder).

Strategy (data-parallel over segments, per sharding hint):
  - batch_ids are sorted, so the host finds the 65 segment boundaries with
    searchsorted and repacks features into a [B, cap, C] array, cast to
    fp16 (rel err ~3.6e-4 through the decoder, far under the 2e-2 gate),
    padded with -65504 (fp16 max-identity).  Each core gets 8 segments.
  - fp16 halves HBM traffic (32MB/core) AND doubles DVE tensor_tensor
    throughput (2x_1P packed mode), so the max-tree (~84us) hides under
    the DMA stream (~89us at the 360 GB/s per-core DMA-engine roofline).
  - The J=2 chunks of each segment stream on BOTH HWDGE queues (SP and
    Act) so queue-side gaps between consecutive DMAs on one ring are
    covered by the other ring (the baseline's single ring left the 16
    DMA engines idle ~29% of the time).
  - Weights/biases (cast to fp16 on host where they feed matmuls) load
    via the SWDGE ring (gpsimd), keeping both HWDGE rings free for the
    feature stream from t=0.
  - Per chunk: tensor_max tree [P, LQ*C] -> [P, RB*C]; combine chunks;
    final tree -> [P, C] fp16; cast to f32, PE-transpose, reduce -> gT.
  - Decoder (fp16 weights, f32 PSUM/biases) runs in two halves: half 0
    at the stream midpoint (hidden), half 1 as the only tail.
"""

import os
import sys
import types

sys.path.insert(0, "/opt/trn_rl_repo")

import numpy as np


def _ensure_axon_hooks():
    """Some images lack antenv.axon_hooks; bass_utils imports it when
    trace=True under axon. Install a shim that lazily wires the real
    ctypes-based NTFF hook from trn_agent_boot if present, else degrades
    to no-trace instead of crashing."""
    try:
        import antenv.axon_hooks  # noqa: F401

        return
    except ImportError:
        pass
    try:
        import antenv
    except ImportError:
        return
    mod = types.ModuleType("antenv.axon_hooks")
    _hook = [None]

    def set_axon_ntff_profile_hook(h):
        _hook[0] = h

    def get_axon_ntff_profile_hook():
        if _hook[0] is None:
            try:
                from trn_agent_boot.trn_boot import _ntff_profile_via_ctypes

                _hook[0] = _ntff_profile_via_ctypes("/opt/axon/libaxon_pjrt.so")
            except Exception:
                return None
        return _hook[0]

    mod.set_axon_ntff_profile_hook = set_axon_ntff_profile_hook
    mod.get_axon_ntff_profile_hook = get_axon_ntff_profile_hook
    sys.modules["antenv.axon_hooks"] = mod
    antenv.axon_hooks = mod

N = 4_194_304
C = 32
B = 64
NUM_POINTS = 1024
NCORES = 8
SPC = B // NCORES  # segments per core
P = 128
J = 2  # DMA chunks per segment (one per HWDGE queue)
NEG = -65504.0  # fp16 lowest: max-identity padding
H1, H2, OUT_D = 256, 512, 3 * NUM_POINTS
K1, K2, NT = H1 // P, H2 // P, OUT_D // 512

LAST_RESULTS = None

_build_cache = {}


def _build(cap):
    if cap in _build_cache:
        return _build_cache[cap]

    import concourse.bacc as bacc
    import concourse.tile as tile
    from concourse import mybir
    from concourse.masks import make_identity
    from contextlib import ExitStack

    L = cap // P  # rows per partition per segment
    LQ = L // J  # rows per partition per DMA chunk
    F = LQ * C  # free elems per chunk tile

    f32 = mybir.dt.float32
    f16 = mybir.dt.float16
    AX = mybir.AxisListType.X
    nc = bacc.Bacc("TRN2", target_bir_lowering=False)

    feats = nc.dram_tensor("feats", [SPC * cap, C], f16, kind="ExternalInput")
    w1 = nc.dram_tensor("w1", [C, H1], f16, kind="ExternalInput")
    b1t = nc.dram_tensor("b1t", [P, K1], f32, kind="ExternalInput")
    w2 = nc.dram_tensor("w2", [H1, H2], f16, kind="ExternalInput")
    b2t = nc.dram_tensor("b2t", [P, K2], f32, kind="ExternalInput")
    w3 = nc.dram_tensor("w3", [H2, OUT_D], f16, kind="ExternalInput")
    b3r = nc.dram_tensor("b3r", [SPC, OUT_D], f32, kind="ExternalInput")
    out = nc.dram_tensor("out", [SPC, OUT_D], f32, kind="ExternalOutput")

    # rows: s*cap + p*L + j*LQ + i  ->  [s, j, p, (i c)]
    fview = feats[:].rearrange("(s p j i) c -> s j p (i c)", s=SPC, p=P, j=J)
    # quarter-chunk view of the same rows, for the last segment's tail
    fview4 = feats[:].rearrange("(s p j i) c -> s j p (i c)", s=SPC, p=P, j=2 * J)

    with ExitStack() as ctx:
        tc = ctx.enter_context(tile.TileContext(nc))
        consts = ctx.enter_context(tc.tile_pool(name="consts", bufs=1))
        fpool = ctx.enter_context(tc.tile_pool(name="feat", bufs=4))  # x J tags
        outp = ctx.enter_context(tc.tile_pool(name="outp", bufs=2))
        redp = ctx.enter_context(tc.tile_pool(name="red", bufs=2 * J))
        ptr = ctx.enter_context(tc.tile_pool(name="ptr", bufs=2, space="PSUM"))
        pmm = ctx.enter_context(tc.tile_pool(name="pmm", bufs=2, space="PSUM"))
        pout = ctx.enter_context(tc.tile_pool(name="pout", bufs=2, space="PSUM"))

        ident = consts.tile([P, P], f32)
        make_identity(nc, ident)

        # Weight/bias tiles: DMAs are emitted AFTER segment 0's feature
        # DMAs (see the segment loop) so the 3.4MB of weights streams
        # BEHIND segment 0 through the shared DMA engines instead of
        # delaying the first tree by ~8us.  They ride the SP ring, whose
        # FIFO then naturally interleaves them between segment 0 and
        # segment 2 (features alternate SP/Act per segment).
        b1_sb = consts.tile([P, K1], f32)
        b2_sb = consts.tile([P, K2], f32)
        HS = SPC // 2  # segments per decoder half
        b3_sb = [
            consts.tile([HS, OUT_D], f32, tag=f"b3h{h}", name=f"b3h{h}")
            for h in range(2)
        ]
        w1_sb = consts.tile([C, H1], f16)
        w2_sb = consts.tile([P, K1, H2], f16)
        w3_sb = consts.tile([P, K2, OUT_D], f16)

        def load_weights():
            nc.sync.dma_start(out=b1_sb, in_=b1t[:])
            nc.sync.dma_start(out=b2_sb, in_=b2t[:])
            for h in range(2):
                nc.sync.dma_start(out=b3_sb[h], in_=b3r[h * HS : (h + 1) * HS])
            nc.sync.dma_start(out=w1_sb, in_=w1[:])
            nc.sync.dma_start(
                out=w2_sb, in_=w2[:].rearrange("(k p) n -> p k n", p=P)
            )
            nc.sync.dma_start(
                out=w3_sb, in_=w3[:].rearrange("(k p) n -> p k n", p=P)
            )

        obs = consts.tile([1, 16], f32)
        gT = consts.tile([C, SPC], f32)
        segobs = consts.tile([1, SPC], f32)

        RB = 8  # row-blocks kept per chunk; small levels are overhead-bound

        def chunk_tree(eng, ft, rj, n0=None):
            # contiguous tree max over the row axis: pairs (i, c) with
            # (i + n/2, c); in-place halving within ft. Stops at RB
            # blocks (tail levels are fixed-overhead-dominated); rj is
            # [P, RB*C] and the cross-chunk combine finishes the job.
            cur = ft
            n = LQ if n0 is None else n0
            while n > 2 * RB:
                if n % 2 == 1:
                    eng.tensor_max(
                        cur[:, 0:C], cur[:, 0:C], cur[:, (n - 1) * C : n * C]
                    )
                    n -= 1
                half = n // 2
                eng.tensor_max(
                    cur[:, 0 : half * C],
                    cur[:, 0 : half * C],
                    cur[:, half * C : n * C],
                )
                n = half
            while n % RB:
                eng.tensor_max(cur[:, 0:C], cur[:, 0:C], cur[:, (n - 1) * C : n * C])
                n -= 1
            eng.tensor_max(
                rj[:, :], cur[:, 0 : (n // 2) * C], cur[:, (n // 2) * C : n * C]
            )

        def decode_half(h):
            # decoder for segments [h*HS, (h+1)*HS): runs while the other
            # half is still streaming, so only the last half is tail time.
            cols = slice(h * HS, (h + 1) * HS)
            # empty segments: reference maps -inf -> 0; padding is -65504,
            # so mask = (g > -60000) in {0,1}; g * mask zeroes empties.
            mask = consts.tile([C, HS], f32, tag=f"mask{h}")
            gfix = consts.tile([C, HS], f32, tag=f"gfix{h}")
            nc.vector.tensor_scalar(
                out=mask[:, :],
                in0=gT[:, cols],
                scalar1=-60000.0,
                scalar2=None,
                op0=mybir.AluOpType.is_gt,
            )
            nc.vector.tensor_mul(gfix[:, :], gT[:, cols], mask[:, :])
            g16 = consts.tile([C, HS], f16, tag=f"g16{h}")
            nc.vector.tensor_copy(out=g16[:, :], in_=gfix[:, :])

            # h1T[m] = relu(W1[:, m]^T @ g + b1[m])   [128, HS] per chunk m
            h1_sb = consts.tile([P, K1, HS], f16, tag=f"h1{h}")
            for m in range(K1):
                pm = pmm.tile([P, HS], f32, tag="pm")
                nc.tensor.matmul(
                    pm[:, :],
                    w1_sb[:, m * P : (m + 1) * P],
                    g16[:, :],
                    start=True,
                    stop=True,
                )
                nc.scalar.activation(
                    out=h1_sb[:, m, :],
                    in_=pm[:, :],
                    func=mybir.ActivationFunctionType.Relu,
                    bias=b1_sb[:, m : m + 1],
                    scale=1.0,
                )

            # h2T[m] = relu(sum_k W2[k, :, m]^T @ h1T[k] + b2[m])
            h2_sb = consts.tile([P, K2, HS], f16, tag=f"h2{h}")
            for m in range(K2):
                pm = pmm.tile([P, HS], f32, tag="pm")
                for k in range(K1):
                    nc.tensor.matmul(
                        pm[:, :],
                        w2_sb[:, k, m * P : (m + 1) * P],
                        h1_sb[:, k, :],
                        start=(k == 0),
                        stop=(k == K1 - 1),
                    )
                nc.scalar.activation(
                    out=h2_sb[:, m, :],
                    in_=pm[:, :],
                    func=mybir.ActivationFunctionType.Relu,
                    bias=b2_sb[:, m : m + 1],
                    scale=1.0,
                )

            # out[:, n] = sum_k h2T[k]^T @ W3[k, :, n] + b3[:, n]
            # streamed per 512-column chunk through a small rotating tile
            for n in range(NT):
                po = pout.tile([HS, 512], f32, tag="po")
                for k in range(K2):
                    nc.tensor.matmul(
                        po[:, :],
                        h2_sb[:, k, :],
                        w3_sb[:, k, n * 512 : (n + 1) * 512],
                        start=(k == 0),
                        stop=(k == K2 - 1),
                    )
                ob = outp.tile([HS, 512], f32, tag="ob")
                nc.vector.tensor_add(
                    ob[:, :],
                    po[:, :],
                    b3_sb[h][:, n * 512 : (n + 1) * 512],
                )
                # SWDGE store: DMASW lanes unused by the feature stream.
                nc.gpsimd.dma_start(
                    out=out[h * HS : (h + 1) * HS, n * 512 : (n + 1) * 512],
                    in_=ob[:, :],
                )

        # Both chunks of segment s ride ONE HWDGE ring, alternating
        # rings per segment: each ring then has a whole 2-segment period
        # (~24us) to retrigger its next DMA, so trigger/semaphore
        # latency never leaves the shared DMA engines idle.
        qeng = [nc.sync, nc.scalar]
        for s in range(SPC):
            q = qeng[s % 2]
            last = s == SPC - 1
            reds = []
            for j in range(J):
                ft = fpool.tile([P, F], f16, tag=f"ft{j}")
                if last and j == J - 1:
                    # Split the final chunk's DMA and tree into halves so
                    # only a quarter-segment of tree work trails the last
                    # byte (the tail tree drops ~2.7us).
                    q.dma_start(out=ft[:, 0 : F // 2], in_=fview4[s, 2 * j])
                    q.dma_start(out=ft[:, F // 2 : F], in_=fview4[s, 2 * j + 1])
                    rj = redp.tile([P, RB * C], f16, tag=f"rj{j}")
                    chunk_tree(nc.vector, ft[:, 0 : F // 2], rj, n0=LQ // 2)
                    rj7 = redp.tile([P, RB * C], f16, tag="rj7")
                    chunk_tree(nc.vector, ft[:, F // 2 : F], rj7, n0=LQ // 2)
                    nc.vector.tensor_max(rj[:, :], rj[:, :], rj7[:, :])
                    reds.append(rj)
                    continue
                q.dma_start(out=ft, in_=fview[s, j])
                rj = redp.tile([P, RB * C], f16, tag=f"rj{j}")
                chunk_tree(nc.vector, ft, rj)
                reds.append(rj)
                if j == 0:
                    # ACT observer: advance Act's DVE clock past the tree
                    # reads of this segment's chunks, covering the slot
                    # releases the NEXT segments' Act-ring reuse-DMAs
                    # depend on -- they then wait only on their own DMA
                    # lane, and the pipeline never drains at segment
                    # boundaries.
                    nc.scalar.copy(
                        out=segobs[0:1, s : s + 1], in_=rj[0:1, 0:1]
                    )
            if s == 0:
                # Weights enter the SP FIFO here -- after segment 0's
                # chunks, before segment 2's -- so they stream behind
                # the first segment instead of ahead of it.
                load_weights()
                # PE (Matmult/LDW) supports only ONE sync wait per
                # instruction, so a matmul whose inputs come from two
                # unobserved semaphores fails to compile. Prime PE with
                # throwaway single-wait ops so it has observed the
                # identity (Pool lane) and the SP weight lane before the
                # real matmuls. Each gets its own PSUM slot (slot reuse
                # would add a second, PE-release wait); the pool closes
                # before first use of the others.
                with tc.tile_pool(name="prime", bufs=1, space="PSUM") as primep:
                    pp = primep.tile([C, P], f32, tag="prime")
                    nc.tensor.transpose(
                        out=pp[0:C, 0:P], in_=ident[:, 0:C], identity=ident[:, :]
                    )
                    # fp16 matmul, both operands from the SP weight lane.
                    pp2 = primep.tile([1, P], f32, tag="prime16")
                    nc.tensor.matmul(
                        pp2[0:1, 0:C],
                        w3_sb[:, 0, 0:1],
                        w3_sb[:, 0, 0:C],
                        start=True,
                        stop=True,
                    )
                # Observers: advance ACT's and DVE's clocks over the SP
                # weight lane (w3 is the LAST weight DMA in the FIFO, so
                # one observer per engine covers all weight/bias loads);
                # decoder relu/add ops then need only their PE wait.
                nc.scalar.copy(out=obs[0:1, 0:1], in_=w3_sb[0:1, 0, 0:1])
                nc.vector.tensor_copy(out=obs[0:1, 3:4], in_=w3_sb[0:1, 0, 0:1])
            # cross-chunk combine (J=2)
            nc.vector.tensor_max(reds[0][:, :], reds[0][:, :], reds[1][:, :])
            rs = reds[0]
            n = RB
            while n > 1:
                half = n // 2
                nc.vector.tensor_max(
                    rs[:, 0 : half * C],
                    rs[:, 0 : half * C],
                    rs[:, half * C : n * C],
                )
                n = half
            rs32 = redp.tile([P, C], f32, tag="rs32")
            nc.vector.tensor_copy(out=rs32[:, :], in_=rs[:, 0:C])
            pt = ptr.tile([C, P], f32, tag="pt")
            nc.tensor.transpose(
                out=pt[:, :], in_=rs32[:, :], identity=ident[:, :]
            )
            nc.vector.reduce_max(out=gT[:, s : s + 1], in_=pt[:, :], axis=AX)
            if s == SPC // 2 - 1:
                decode_half(0)

        decode_half(1)
    nc.compile()
    _build_cache[cap] = nc
    return nc


def kernel(**inputs):
    global LAST_RESULTS
    features = np.asarray(inputs["features"], dtype=np.float32)
    batch_ids = np.asarray(inputs["batch_ids"])
    W1 = np.asarray(inputs["W1"], dtype=np.float32)
    b1 = np.asarray(inputs["b1"], dtype=np.float32)
    W2 = np.asarray(inputs["W2"], dtype=np.float32)
    b2 = np.asarray(inputs["b2"], dtype=np.float32)
    W3 = np.asarray(inputs["W3"], dtype=np.float32)
    b3 = np.asarray(inputs["b3"], dtype=np.float32)

    bounds = np.searchsorted(batch_ids, np.arange(B + 1), side="left")
    seg_len = np.diff(bounds)
    maxlen = max(1, int(seg_len.max()))
    L = -(-maxlen // P)  # ceil
    L = -(-L // (2 * J)) * (2 * J)  # multiple of 2J (quarter-chunk view)
    L = max(L, 64)  # keep LQ//2 >= 2*RB so the tree structure holds
    cap = L * P

    packed = np.empty((B, cap, C), np.float16)
    for b in range(B):
        lo, hi = int(bounds[b]), int(bounds[b + 1])
        n = hi - lo
        packed[b, :n] = features[lo:hi]
        packed[b, n:] = NEG

    w1h = np.ascontiguousarray(W1.astype(np.float16))
    w2h = np.ascontiguousarray(W2.astype(np.float16))
    w3h = np.ascontiguousarray(W3.astype(np.float16))
    b1t = np.ascontiguousarray(b1.reshape(K1, P).T)
    b2t = np.ascontiguousarray(b2.reshape(K2, P).T)
    b3r = np.ascontiguousarray(np.broadcast_to(b3, (SPC, OUT_D)))

    nc = _build(cap)

    in_maps = []
    for d in range(NCORES):
        in_maps.append(
            {
                "feats": packed[d * SPC : (d + 1) * SPC].reshape(SPC * cap, C),
                "w1": w1h,
                "b1t": b1t,
                "w2": w2h,
                "b2t": b2t,
                "w3": w3h,
                "b3r": b3r,
            }
        )

    _ensure_axon_hooks()
    from concourse.bass_utils import run_bass_kernel_spmd

    core_ids = list(range(NCORES))
    try:
        res = run_bass_kernel_spmd(nc, in_maps, core_ids=core_ids)
    except Exception:
        if os.environ.get("BASS_TRACE") and not os.environ.get("BASS_NEVER_TRACE"):
            # trace post-processing can fail in restricted containers;
            # retry without tracing so the numeric result still lands.
            os.environ["BASS_NEVER_TRACE"] = "1"
            try:
                res = run_bass_kernel_spmd(nc, in_maps, core_ids=core_ids)
            finally:
                os.environ.pop("BASS_NEVER_TRACE", None)
        else:
            raise
    LAST_RESULTS = res

    full = np.concatenate([r["out"] for r in res.results], axis=0)
    return full.reshape(B, 3, NUM_POINTS)


# revision 14
# speedup vs baseline: 1.1349x; 1.1349x over previous
"""Trainium2 Bass kernel for nn_FCGFAutoencoder (segment_max -> 3-layer MLP decoder).

Strategy (data-parallel over segments, per sharding hint):
  - batch_ids are sorted, so the host finds the 65 segment boundaries with
    searchsorted and repacks features into a [B, cap, C] array, cast to
    fp16 (rel err ~3.6e-4 through the decoder, far under the 2e-2 gate),
    padded with -65504 (fp16 max-identity).  Each core gets 8 segments.
  - fp16 halves HBM traffic (32MB/core) AND doubles DVE tensor_tensor
    throughput (2x_1P packed mode), so the max-tree (~84us) hides under
    the DMA stream (~89us at the 360 GB/s per-core DMA-engine roofline).
  - The J=2 chunks of each segment stream on BOTH HWDGE queues (SP and
    Act) so queue-side gaps between consecutive DMAs on one ring are
    covered by the other ring (the baseline's single ring left the 16
    DMA engines idle ~29% of the time).
  - Weights/biases (cast to fp16 on host where they feed matmuls) load
    via the SWDGE ring (gpsimd), keeping both HWDGE rings free for the
    feature stream from t=0.
  - Per chunk: tensor_max tree [P, LQ*C] -> [P, RB*C]; combine chunks;
    final tree -> [P, C] fp16; cast to f32, PE-transpose, reduce -> gT.
  - Decoder (fp16 weights, f32 PSUM/biases) runs in two halves: half 0
    at the stream midpoint (hidden), half 1 as the only tail.
"""

import os
import sys
import types

sys.path.insert(0, "/opt/trn_rl_repo")

import numpy as np


def _ensure_axon_hooks():
    """Some images lack antenv.axon_hooks; bass_utils imports it when
    trace=True under axon. Install a shim that lazily wires the real
    ctypes-based NTFF hook from trn_agent_boot if present, else degrades
    to no-trace instead of crashing."""
    try:
        import antenv.axon_hooks  # noqa: F401

        return
    except ImportError:
        pass
    try:
        import antenv
    except ImportError:
        return
    mod = types.ModuleType("antenv.axon_hooks")
    _hook = [None]

    def set_axon_ntff_profile_hook(h):
        _hook[0] = h

    def get_axon_ntff_profile_hook():
        if _hook[0] is None:
            try:
                from trn_agent_boot.trn_boot import _ntff_profile_via_ctypes

                _hook[0] = _ntff_profile_via_ctypes("/opt/axon/libaxon_pjrt.so")
            except Exception:
                return None
        return _hook[0]

    mod.set_axon_ntff_profile_hook = set_axon_ntff_profile_hook
    mod.get_axon_ntff_profile_hook = get_axon_ntff_profile_hook
    sys.modules["antenv.axon_hooks"] = mod
    antenv.axon_hooks = mod

N = 4_194_304
C = 32
B = 64
NUM_POINTS = 1024
NCORES = 8
SPC = B // NCORES  # segments per core
P = 128
J = 2  # DMA chunks per segment (one per HWDGE queue)
NEG = -65504.0  # fp16 lowest: max-identity padding
H1, H2, OUT_D = 256, 512, 3 * NUM_POINTS
K1, K2, NT = H1 // P, H2 // P, OUT_D // 512

LAST_RESULTS = None

_build_cache = {}


def _build(cap):
    if cap in _build_cache:
        return _build_cache[cap]

    import concourse.bacc as bacc
    import concourse.tile as tile
    from concourse import mybir
    from concourse.masks import make_identity
    from contextlib import ExitStack

    L = cap // P  # rows per partition per segment
    LQ = L // J  # rows per partition per DMA chunk
    F = LQ * C  # free elems per chunk tile

    f32 = mybir.dt.float32
    f16 = mybir.dt.float16
    AX = mybir.AxisListType.X
    nc = bacc.Bacc("TRN2", target_bir_lowering=False)

    feats = nc.dram_tensor("feats", [SPC * cap, C], f16, kind="ExternalInput")
    w1 = nc.dram_tensor("w1", [C, H1], f16, kind="ExternalInput")
    b1t = nc.dram_tensor("b1t", [P, K1], f32, kind="ExternalInput")
    w2 = nc.dram_tensor("w2", [H1, H2], f16, kind="ExternalInput")
    b2t = nc.dram_tensor("b2t", [P, K2], f32, kind="ExternalInput")
    w3 = nc.dram_tensor("w3", [H2, OUT_D], f16, kind="ExternalInput")
    b3r = nc.dram_tensor("b3r", [SPC, OUT_D], f32, kind="ExternalInput")
    out = nc.dram_tensor("out", [SPC, OUT_D], f32, kind="ExternalOutput")

    # rows: s*cap + p*L + j*LQ + i  ->  [s, j, p, (i c)]
    fview = feats[:].rearrange("(s p j i) c -> s j p (i c)", s=SPC, p=P, j=J)
    # quarter-chunk view of the same rows, for the last segment's tail
    fview4 = feats[:].rearrange("(s p j i) c -> s j p (i c)", s=SPC, p=P, j=2 * J)

    with ExitStack() as ctx:
        tc = ctx.enter_context(tile.TileContext(nc))
        consts = ctx.enter_context(tc.tile_pool(name="consts", bufs=1))
        fpool = ctx.enter_context(tc.tile_pool(name="feat", bufs=4))  # x J tags
        outp = ctx.enter_context(tc.tile_pool(name="outp", bufs=2))
        redp = ctx.enter_context(tc.tile_pool(name="red", bufs=2 * J))
        ptr = ctx.enter_context(tc.tile_pool(name="ptr", bufs=2, space="PSUM"))
        pmm = ctx.enter_context(tc.tile_pool(name="pmm", bufs=2, space="PSUM"))
        pout = ctx.enter_context(tc.tile_pool(name="pout", bufs=2, space="PSUM"))

        ident = consts.tile([P, P], f32)
        make_identity(nc, ident)

        # Weight/bias tiles: DMAs are emitted AFTER segment 0's feature
        # DMAs (see the segment loop) so the 3.4MB of weights streams
        # BEHIND segment 0 through the shared DMA engines instead of
        # delaying the first tree by ~8us.  They ride the SP ring, whose
        # FIFO then naturally interleaves them between segment 0 and
        # segment 2 (features alternate SP/Act per segment).
        b1_sb = consts.tile([P, K1], f32)
        b2_sb = consts.tile([P, K2], f32)
        HS = SPC // 2  # segments per decoder half
        b3_sb = [
            consts.tile([HS, OUT_D], f32, tag=f"b3h{h}", name=f"b3h{h}")
            for h in range(2)
        ]
        w1_sb = consts.tile([C, H1], f16)
        w2_sb = consts.tile([P, K1, H2], f16)
        w3_sb = consts.tile([P, K2, OUT_D], f16)

        def load_weights():
            nc.sync.dma_start(out=b1_sb, in_=b1t[:])
            nc.sync.dma_start(out=b2_sb, in_=b2t[:])
            for h in range(2):
                nc.sync.dma_start(out=b3_sb[h], in_=b3r[h * HS : (h + 1) * HS])
            nc.sync.dma_start(out=w1_sb, in_=w1[:])
            nc.sync.dma_start(
                out=w2_sb, in_=w2[:].rearrange("(k p) n -> p k n", p=P)
            )
            nc.sync.dma_start(
                out=w3_sb, in_=w3[:].rearrange("(k p) n -> p k n", p=P)
            )

        obs = consts.tile([1, 16], f32)
        gT = consts.tile([C, SPC], f32)

        RB = 8  # row-blocks kept per chunk; small levels are overhead-bound

        def chunk_tree(eng, ft, rj, n0=None):
            # contiguous tree max over the row axis: pairs (i, c) with
            # (i + n/2, c); in-place halving within ft. Stops at RB
            # blocks (tail levels are fixed-overhead-dominated); rj is
            # [P, RB*C] and the cross-chunk combine finishes the job.
            cur = ft
            n = LQ if n0 is None else n0
            while n > 2 * RB:
                if n % 2 == 1:
                    eng.tensor_max(
                        cur[:, 0:C], cur[:, 0:C], cur[:, (n - 1) * C : n * C]
                    )
                    n -= 1
                half = n // 2
                eng.tensor_max(
                    cur[:, 0 : half * C],
                    cur[:, 0 : half * C],
                    cur[:, half * C : n * C],
                )
                n = half
            while n % RB:
                eng.tensor_max(cur[:, 0:C], cur[:, 0:C], cur[:, (n - 1) * C : n * C])
                n -= 1
            eng.tensor_max(
                rj[:, :], cur[:, 0 : (n // 2) * C], cur[:, (n // 2) * C : n * C]
            )

        def decode_half(h):
            # decoder for segments [h*HS, (h+1)*HS): runs while the other
            # half is still streaming, so only the last half is tail time.
            cols = slice(h * HS, (h + 1) * HS)
            # empty segments: reference maps -inf -> 0; padding is -65504,
            # so mask = (g > -60000) in {0,1}; g * mask zeroes empties.
            mask = consts.tile([C, HS], f32, tag=f"mask{h}")
            gfix = consts.tile([C, HS], f32, tag=f"gfix{h}")
            nc.vector.tensor_scalar(
                out=mask[:, :],
                in0=gT[:, cols],
                scalar1=-60000.0,
                scalar2=None,
                op0=mybir.AluOpType.is_gt,
            )
            nc.vector.tensor_mul(gfix[:, :], gT[:, cols], mask[:, :])
            g16 = consts.tile([C, HS], f16, tag=f"g16{h}")
            nc.vector.tensor_copy(out=g16[:, :], in_=gfix[:, :])

            # h1T[m] = relu(W1[:, m]^T @ g + b1[m])   [128, HS] per chunk m
            h1_sb = consts.tile([P, K1, HS], f16, tag=f"h1{h}")
            for m in range(K1):
                pm = pmm.tile([P, HS], f32, tag="pm")
                nc.tensor.matmul(
                    pm[:, :],
                    w1_sb[:, m * P : (m + 1) * P],
                    g16[:, :],
                    start=True,
                    stop=True,
                )
                nc.scalar.activation(
                    out=h1_sb[:, m, :],
                    in_=pm[:, :],
                    func=mybir.ActivationFunctionType.Relu,
                    bias=b1_sb[:, m : m + 1],
                    scale=1.0,
                )

            # h2T[m] = relu(sum_k W2[k, :, m]^T @ h1T[k] + b2[m])
            h2_sb = consts.tile([P, K2, HS], f16, tag=f"h2{h}")
            for m in range(K2):
                pm = pmm.tile([P, HS], f32, tag="pm")
                for k in range(K1):
                    nc.tensor.matmul(
                        pm[:, :],
                        w2_sb[:, k, m * P : (m + 1) * P],
                        h1_sb[:, k, :],
                        start=(k == 0),
                        stop=(k == K1 - 1),
                    )
                nc.scalar.activation(
                    out=h2_sb[:, m, :],
                    in_=pm[:, :],
                    func=mybir.ActivationFunctionType.Relu,
                    bias=b2_sb[:, m : m + 1],
                    scale=1.0,
                )

            # out[:, n] = sum_k h2T[k]^T @ W3[k, :, n] + b3[:, n]
            # streamed per 512-column chunk through a small rotating tile
            for n in range(NT):
                po = pout.tile([HS, 512], f32, tag="po")
                for k in range(K2):
                    nc.tensor.matmul(
                        po[:, :],
                        h2_sb[:, k, :],
                        w3_sb[:, k, n * 512 : (n + 1) * 512],
                        start=(k == 0),
                        stop=(k == K2 - 1),
                    )
                ob = outp.tile([HS, 512], f32, tag="ob")
                nc.vector.tensor_add(
                    ob[:, :],
                    po[:, :],
                    b3_sb[h][:, n * 512 : (n + 1) * 512],
                )
                # SWDGE store: DMASW lanes unused by the feature stream.
                nc.gpsimd.dma_start(
                    out=out[h * HS : (h + 1) * HS, n * 512 : (n + 1) * 512],
                    in_=ob[:, :],
                )

        # Phase 1 -- emit EVERY feature DMA trigger before any compute
        # op enters either HWDGE ring's sequencer FIFO.  A sequencer
        # executes its FIFO in order, so any compute/observer op that
        # waits on DVE progress would stall all later DMA triggers
        # behind it and make the stream DVE-paced (the baseline's ~29%
        # DMA idle).  Emitted first, each trigger carries at most its
        # one slot-release wait (bufs=4 deep, ~4 segments of slack) and
        # the rings stay saturated end to end.  Both chunks of segment
        # s ride ONE ring, alternating rings per segment.
        qeng = [nc.sync, nc.scalar]
        fts = []
        for s in range(SPC):
            q = qeng[s % 2]
            last = s == SPC - 1
            pair = []
            for j in range(J):
                ft = fpool.tile([P, F], f16, tag=f"ft{j}", name=f"ft{s}_{j}")
                if last and j == J - 1:
                    # Split the final chunk's DMA so only a quarter
                    # segment of tree work trails the last byte.
                    q.dma_start(out=ft[:, 0 : F // 2], in_=fview4[s, 2 * j])
                    q.dma_start(out=ft[:, F // 2 : F], in_=fview4[s, 2 * j + 1])
                else:
                    q.dma_start(out=ft, in_=fview[s, j])
                pair.append(ft)
            fts.append(pair)
            if s == 0:
                # Weights enter the SP FIFO here -- after segment 0's
                # chunks, before segment 2's -- so they stream behind
                # the first segment instead of ahead of it.
                load_weights()

        # PE (Matmult/LDW) supports only ONE sync wait per instruction,
        # so a matmul whose inputs come from two unobserved semaphores
        # fails to compile. Prime PE with throwaway single-wait ops so
        # it has observed the identity (Pool lane) and the SP weight
        # lane before the real matmuls. Each gets its own PSUM slot
        # (slot reuse would add a second, PE-release wait); the pool
        # closes before first use of the others.
        with tc.tile_pool(name="prime", bufs=1, space="PSUM") as primep:
            pp = primep.tile([C, P], f32, tag="prime")
            nc.tensor.transpose(
                out=pp[0:C, 0:P], in_=ident[:, 0:C], identity=ident[:, :]
            )
            # fp16 matmul, both operands from the SP weight lane.
            pp2 = primep.tile([1, P], f32, tag="prime16")
            nc.tensor.matmul(
                pp2[0:1, 0:C],
                w3_sb[:, 0, 0:1],
                w3_sb[:, 0, 0:C],
                start=True,
                stop=True,
            )
        # Observers: advance ACT's and DVE's clocks over the SP weight
        # lane (w3 is the LAST weight DMA in the FIFO, so one observer
        # per engine covers all weight/bias loads); decoder relu/add
        # ops then need only their PE wait.  These sit AFTER all DMA
        # triggers in the Act FIFO, so they stall nothing.
        nc.scalar.copy(out=obs[0:1, 0:1], in_=w3_sb[0:1, 0, 0:1])
        nc.vector.tensor_copy(out=obs[0:1, 3:4], in_=w3_sb[0:1, 0, 0:1])

        # Phase 2 -- the reduction trees and decoder.
        for s in range(SPC):
            last = s == SPC - 1
            reds = []
            for j in range(J):
                ft = fts[s][j]
                if last and j == J - 1:
                    rj = redp.tile([P, RB * C], f16, tag=f"rj{j}")
                    chunk_tree(nc.vector, ft[:, 0 : F // 2], rj, n0=LQ // 2)
                    rj7 = redp.tile([P, RB * C], f16, tag="rj7")
                    chunk_tree(nc.vector, ft[:, F // 2 : F], rj7, n0=LQ // 2)
                    nc.vector.tensor_max(rj[:, :], rj[:, :], rj7[:, :])
                else:
                    rj = redp.tile([P, RB * C], f16, tag=f"rj{j}")
                    chunk_tree(nc.vector, ft, rj)
                reds.append(rj)
            # cross-chunk combine (J=2)
            nc.vector.tensor_max(reds[0][:, :], reds[0][:, :], reds[1][:, :])
            rs = reds[0]
            n = RB
            while n > 1:
                half = n // 2
                nc.vector.tensor_max(
                    rs[:, 0 : half * C],
                    rs[:, 0 : half * C],
                    rs[:, half * C : n * C],
                )
                n = half
            rs32 = redp.tile([P, C], f32, tag="rs32")
            nc.vector.tensor_copy(out=rs32[:, :], in_=rs[:, 0:C])
            pt = ptr.tile([C, P], f32, tag="pt")
            nc.tensor.transpose(
                out=pt[:, :], in_=rs32[:, :], identity=ident[:, :]
            )
            nc.vector.reduce_max(out=gT[:, s : s + 1], in_=pt[:, :], axis=AX)
            if s == SPC // 2 - 1:
                decode_half(0)

        decode_half(1)
    nc.compile()
    _build_cache[cap] = nc
    return nc


def kernel(**inputs):
    global LAST_RESULTS
    features = np.asarray(inputs["features"], dtype=np.float32)
    batch_ids = np.asarray(inputs["batch_ids"])
    W1 = np.asarray(inputs["W1"], dtype=np.float32)
    b1 = np.asarray(inputs["b1"], dtype=np.float32)
    W2 = np.asarray(inputs["W2"], dtype=np.float32)
    b2 = np.asarray(inputs["b2"], dtype=np.float32)
    W3 = np.asarray(inputs["W3"], dtype=np.float32)
    b3 = np.asarray(inputs["b3"], dtype=np.float32)

    bounds = np.searchsorted(batch_ids, np.arange(B + 1), side="left")
    seg_len = np.diff(bounds)
    maxlen = max(1, int(seg_len.max()))
    L = -(-maxlen // P)  # ceil
    L = -(-L // (2 * J)) * (2 * J)  # multiple of 2J (quarter-chunk view)
    L = max(L, 64)  # keep LQ//2 >= 2*RB so the tree structure holds
    cap = L * P

    packed = np.empty((B, cap, C), np.float16)
    for b in range(B):
        lo, hi = int(bounds[b]), int(bounds[b + 1])
        n = hi - lo
        packed[b, :n] = features[lo:hi]
        packed[b, n:] = NEG

    w1h = np.ascontiguousarray(W1.astype(np.float16))
    w2h = np.ascontiguousarray(W2.astype(np.float16))
    w3h = np.ascontiguousarray(W3.astype(np.float16))
    b1t = np.ascontiguousarray(b1.reshape(K1, P).T)
    b2t = np.ascontiguousarray(b2.reshape(K2, P).T)
    b3r = np.ascontiguousarray(np.broadcast_to(b3, (SPC, OUT_D)))

    nc = _build(cap)

    in_maps = []
    for d in range(NCORES):
        in_maps.append(
            {
                "feats": packed[d * SPC : (d + 1) * SPC].reshape(SPC * cap, C),
                "w1": w1h,
                "b1t": b1t,
                "w2": w2h,
                "b2t": b2t,
                "w3": w3h,
                "b3r": b3r,
            }
        )

    _ensure_axon_hooks()
    from concourse.bass_utils import run_bass_kernel_spmd

    core_ids = list(range(NCORES))
    try:
        res = run_bass_kernel_spmd(nc, in_maps, core_ids=core_ids)
    except Exception:
        if os.environ.get("BASS_TRACE") and not os.environ.get("BASS_NEVER_TRACE"):
            # trace post-processing can fail in restricted containers;
            # retry without tracing so the numeric result still lands.
            os.environ["BASS_NEVER_TRACE"] = "1"
            try:
                res = run_bass_kernel_spmd(nc, in_maps, core_ids=core_ids)
            finally:
                os.environ.pop("BASS_NEVER_TRACE", None)
        else:
            raise
    LAST_RESULTS = res

    full = np.concatenate([r["out"] for r in res.results], axis=0)
    return full.reshape(B, 3, NUM_POINTS)


# revision 15
# speedup vs baseline: 1.1992x; 1.0567x over previous
"""Trainium2 Bass kernel for nn_FCGFAutoencoder (segment_max -> 3-layer MLP decoder).

Strategy (data-parallel over segments, per sharding hint):
  - batch_ids are sorted, so the host finds the 65 segment boundaries with
    searchsorted and repacks features into a [B, cap, C] array, cast to
    fp16 (rel err ~7e-4 through the decoder, far under the 2e-2 gate),
    padded with -65504 (fp16 max-identity).  Each core gets 8 segments.
  - fp16 halves HBM traffic (32MB/core) AND doubles DVE tensor_tensor
    throughput (2x_1P packed mode), so the max-tree (~85us) tracks the
    DMA stream (~93us at the 360 GB/s per-core DMA-engine roofline).
  - ALL feature DMA triggers are emitted before any compute op enters
    either HWDGE ring's sequencer FIFO (a sequencer executes its FIFO in
    order, so a compute op waiting on DVE would stall every later DMA
    trigger behind it and make the stream DVE-paced).  One whole-segment
    DMA per segment (J=1, fewer DVE ops), segments alternating between
    the SP and Act rings; segment 7 is quarter-split so only ~1/4
    segment of tree work trails the last feature byte.
  - Weight schedule: small weights (b*, w1, w2) ride the SP ring right
    behind segment 0; W3 (3.1MB, only needed by the decoder tail) rides
    the Act ring BEHIND segment 7, split into 6 column-chunk tiles the
    out-layer consumes as they land.  Features therefore stream
    back-to-back and the decoder is never weight-gated.
  - Per segment: tensor_max tree [P, L*C] -> [P, RB*C] -> [P, C] fp16;
    cast to f32, PE-transpose, DVE reduce -> gT column.
  - Decoder runs ONCE over all 8 segments after the last tree (its cost
    is dominated by streaming W2/W3 columns through the PE, so one full
    decode costs the same as a half); b3 is folded in as a rank-1
    ones x b3 matmul on the PE and ACT moves PSUM->SBUF, keeping the
    DVE off the tail's critical path.
"""

import os
import sys
import types

sys.path.insert(0, "/opt/trn_rl_repo")

import numpy as np


def _ensure_axon_hooks():
    """Some images lack antenv.axon_hooks; bass_utils imports it when
    trace=True under axon. Install a shim that lazily wires the real
    ctypes-based NTFF hook from trn_agent_boot if present, else degrades
    to no-trace instead of crashing."""
    try:
        import antenv.axon_hooks  # noqa: F401

        return
    except ImportError:
        pass
    try:
        import antenv
    except ImportError:
        return
    mod = types.ModuleType("antenv.axon_hooks")
    _hook = [None]

    def set_axon_ntff_profile_hook(h):
        _hook[0] = h

    def get_axon_ntff_profile_hook():
        if _hook[0] is None:
            try:
                from trn_agent_boot.trn_boot import _ntff_profile_via_ctypes

                _hook[0] = _ntff_profile_via_ctypes("/opt/axon/libaxon_pjrt.so")
            except Exception:
                return None
        return _hook[0]

    mod.set_axon_ntff_profile_hook = set_axon_ntff_profile_hook
    mod.get_axon_ntff_profile_hook = get_axon_ntff_profile_hook
    sys.modules["antenv.axon_hooks"] = mod
    antenv.axon_hooks = mod

N = 4_194_304
C = 32
B = 64
NUM_POINTS = 1024
NCORES = 8
SPC = B // NCORES  # segments per core
P = 128
NEG = -65504.0  # fp16 lowest: max-identity padding
H1, H2, OUT_D = 256, 512, 3 * NUM_POINTS
K1, K2, NT = H1 // P, H2 // P, OUT_D // 512

LAST_RESULTS = None

_build_cache = {}


def _build(cap):
    if cap in _build_cache:
        return _build_cache[cap]

    import concourse.bacc as bacc
    import concourse.tile as tile
    from concourse import mybir
    from concourse.masks import make_identity
    from contextlib import ExitStack

    L = cap // P  # rows per partition per segment
    F = L * C  # free elems per segment tile
    LQ4 = L // 4  # rows per quarter chunk (segment 7 tail split)

    f32 = mybir.dt.float32
    f16 = mybir.dt.float16
    AX = mybir.AxisListType.X
    nc = bacc.Bacc("TRN2", target_bir_lowering=False)

    feats = nc.dram_tensor("feats", [SPC * cap, C], f16, kind="ExternalInput")
    w1 = nc.dram_tensor("w1", [C, H1], f16, kind="ExternalInput")
    b1t = nc.dram_tensor("b1t", [P, K1], f32, kind="ExternalInput")
    w2 = nc.dram_tensor("w2", [H1, H2], f16, kind="ExternalInput")
    b2t = nc.dram_tensor("b2t", [P, K2], f32, kind="ExternalInput")
    w3 = nc.dram_tensor("w3", [H2, OUT_D], f16, kind="ExternalInput")
    b3t = nc.dram_tensor("b3t", [1, OUT_D], f16, kind="ExternalInput")
    out = nc.dram_tensor("out", [SPC, OUT_D], f32, kind="ExternalOutput")

    # rows: s*cap + p*L + i  ->  [s, p, (i c)]
    fview = feats[:].rearrange("(s p i) c -> s p (i c)", s=SPC, p=P)
    # quarter-chunk view of the same rows, for the last segment's tail
    fview4 = feats[:].rearrange("(s p j i) c -> s j p (i c)", s=SPC, p=P, j=4)
    w3view = w3[:].rearrange("(k p) n -> p k n", p=P)

    with ExitStack() as ctx:
        tc = ctx.enter_context(tile.TileContext(nc))
        consts = ctx.enter_context(tc.tile_pool(name="consts", bufs=1))
        fpool = ctx.enter_context(tc.tile_pool(name="feat", bufs=4))
        outp = ctx.enter_context(tc.tile_pool(name="outp", bufs=2))
        redp = ctx.enter_context(tc.tile_pool(name="red", bufs=4))
        ptr = ctx.enter_context(tc.tile_pool(name="ptr", bufs=2, space="PSUM"))
        pmm = ctx.enter_context(tc.tile_pool(name="pmm", bufs=2, space="PSUM"))
        pout = ctx.enter_context(tc.tile_pool(name="pout", bufs=2, space="PSUM"))

        ones = consts.tile([1, SPC], f16)
        nc.gpsimd.memset(ones, 1.0)
        ident = consts.tile([P, P], f32)
        make_identity(nc, ident)

        b1_sb = consts.tile([P, K1], f32)
        b2_sb = consts.tile([P, K2], f32)
        b3_sb = consts.tile([1, OUT_D], f16)
        w1_sb = consts.tile([C, H1], f16)
        w2_sb = consts.tile([P, K1, H2], f16)
        w3_sb = [
            consts.tile([P, K2, 512], f16, tag=f"w3c{n}", name=f"w3c{n}")
            for n in range(NT)
        ]

        obs = consts.tile([1, 16], f32)
        gT = consts.tile([C, SPC], f32)

        RB = 8  # row-blocks kept per chunk; small levels are overhead-bound

        def chunk_tree(eng, ft, rj, n0):
            # contiguous tree max over the row axis: pairs (i, c) with
            # (i + n/2, c); in-place halving within ft. Stops at RB
            # blocks (tail levels are fixed-overhead-dominated).
            cur = ft
            n = n0
            while n > 2 * RB:
                if n % 2 == 1:
                    eng.tensor_max(
                        cur[:, 0:C], cur[:, 0:C], cur[:, (n - 1) * C : n * C]
                    )
                    n -= 1
                half = n // 2
                eng.tensor_max(
                    cur[:, 0 : half * C],
                    cur[:, 0 : half * C],
                    cur[:, half * C : n * C],
                )
                n = half
            while n % RB:
                eng.tensor_max(cur[:, 0:C], cur[:, 0:C], cur[:, (n - 1) * C : n * C])
                n -= 1
            eng.tensor_max(
                rj[:, :], cur[:, 0 : (n // 2) * C], cur[:, (n // 2) * C : n * C]
            )

        # ---- Phase 1: every DMA trigger, in ring-FIFO order ----------
        qeng = [nc.sync, nc.scalar]
        fts = []
        for s in range(SPC):
            q = qeng[s % 2]
            ft = fpool.tile([P, F], f16, tag="ft", name=f"ft{s}")
            if s == SPC - 1:
                for j4 in range(4):
                    q.dma_start(
                        out=ft[:, j4 * (F // 4) : (j4 + 1) * (F // 4)],
                        in_=fview4[s, j4],
                    )
            else:
                q.dma_start(out=ft, in_=fview[s])
            fts.append(ft)
            if s == 0:
                # Small weights enter the SP FIFO here -- after segment
                # 0, before segment 2 -- streaming behind the first
                # segment instead of ahead of it.
                nc.sync.dma_start(out=b1_sb, in_=b1t[:])
                nc.sync.dma_start(out=b2_sb, in_=b2t[:])
                nc.sync.dma_start(out=b3_sb, in_=b3t[:])
                nc.sync.dma_start(out=w1_sb, in_=w1[:])
                nc.sync.dma_start(
                    out=w2_sb, in_=w2[:].rearrange("(k p) n -> p k n", p=P)
                )
        # W3 column chunks ride the Act ring BEHIND segment 7: the
        # decoder tail consumes them as they land, features never wait.
        for n in range(NT):
            nc.scalar.dma_start(
                out=w3_sb[n], in_=w3view[:, :, n * 512 : (n + 1) * 512]
            )

        # ---- PE priming (one-wait rule) ------------------------------
        # PE supports only ONE sync wait per instruction; prime it with
        # throwaway single-wait ops so it has observed the identity
        # (Pool lane) and the SP weight lane before the real matmuls.
        with tc.tile_pool(name="prime", bufs=1, space="PSUM") as primep:
            pp = primep.tile([C, P], f32, tag="prime")
            nc.tensor.transpose(
                out=pp[0:C, 0:P], in_=ident[:, 0:C], identity=ident[:, :]
            )
            # fp16 matmul, both operands from the SP weight lane.
            pp2 = primep.tile([1, P], f32, tag="prime16")
            nc.tensor.matmul(
                pp2[0:1, 0:C],
                w1_sb[:, 0:1],
                w1_sb[:, 0:C],
                start=True,
                stop=True,
            )
        # ACT observer over the SP weight lane (w2 is the last small
        # weight in the SP FIFO): decoder relus then carry only their
        # PE wait.  Sits after all Act-ring DMA triggers, stalls nothing.
        nc.scalar.copy(out=obs[0:1, 0:1], in_=w2_sb[0:1, 0, 0:1])

        # ---- Phase 2: reduction trees --------------------------------
        for s in range(SPC):
            ft = fts[s]
            if s == SPC - 1:
                rj = redp.tile([P, RB * C], f16, tag="rj")
                chunk_tree(nc.vector, ft[:, 0 : F // 4], rj, LQ4)
                for j4 in range(1, 4):
                    qrj = redp.tile(
                        [P, RB * C], f16, tag=f"qrj{j4}", name=f"qrj{j4}", bufs=1
                    )
                    chunk_tree(
                        nc.vector,
                        ft[:, j4 * (F // 4) : (j4 + 1) * (F // 4)],
                        qrj,
                        LQ4,
                    )
                    nc.vector.tensor_max(rj[:, :], rj[:, :], qrj[:, :])
            else:
                rj = redp.tile([P, RB * C], f16, tag="rj")
                chunk_tree(nc.vector, ft, rj, L)
            n = RB
            while n > 1:
                half = n // 2
                nc.vector.tensor_max(
                    rj[:, 0 : half * C],
                    rj[:, 0 : half * C],
                    rj[:, half * C : n * C],
                )
                n = half
            rs32 = redp.tile([P, C], f32, tag="rs32")
            nc.vector.tensor_copy(out=rs32[:, :], in_=rj[:, 0:C])
            pt = ptr.tile([C, P], f32, tag="pt")
            nc.tensor.transpose(
                out=pt[:, :], in_=rs32[:, :], identity=ident[:, :]
            )
            nc.vector.reduce_max(out=gT[:, s : s + 1], in_=pt[:, :], axis=AX)

        # ---- Decoder (all 8 segments at once) ------------------------
        # empty segments: reference maps -inf -> 0; padding is -65504,
        # so mask = (g > -60000) in {0,1}; g * mask zeroes empties.
        mask = consts.tile([C, SPC], f32)
        gfix = consts.tile([C, SPC], f32)
        nc.vector.tensor_scalar(
            out=mask[:, :],
            in0=gT[:, :],
            scalar1=-60000.0,
            scalar2=None,
            op0=mybir.AluOpType.is_gt,
        )
        nc.vector.tensor_mul(gfix[:, :], gT[:, :], mask[:, :])
        g16 = consts.tile([C, SPC], f16)
        nc.vector.tensor_copy(out=g16[:, :], in_=gfix[:, :])

        # h1T[m] = relu(W1[:, m]^T @ g + b1[m])   [128, SPC] per chunk m
        h1_sb = consts.tile([P, K1, SPC], f16)
        for m in range(K1):
            pm = pmm.tile([P, SPC], f32, tag="pm")
            nc.tensor.matmul(
                pm[:, :],
                w1_sb[:, m * P : (m + 1) * P],
                g16[:, :],
                start=True,
                stop=True,
            )
            nc.scalar.activation(
                out=h1_sb[:, m, :],
                in_=pm[:, :],
                func=mybir.ActivationFunctionType.Relu,
                bias=b1_sb[:, m : m + 1],
                scale=1.0,
            )

        # h2T[m] = relu(sum_k W2[k, :, m]^T @ h1T[k] + b2[m])
        h2_sb = consts.tile([P, K2, SPC], f16)
        for m in range(K2):
            pm = pmm.tile([P, SPC], f32, tag="pm")
            for k in range(K1):
                nc.tensor.matmul(
                    pm[:, :],
                    w2_sb[:, k, m * P : (m + 1) * P],
                    h1_sb[:, k, :],
                    start=(k == 0),
                    stop=(k == K1 - 1),
                )
            nc.scalar.activation(
                out=h2_sb[:, m, :],
                in_=pm[:, :],
                func=mybir.ActivationFunctionType.Relu,
                bias=b2_sb[:, m : m + 1],
                scale=1.0,
            )

        # out[:, n] = sum_k h2T[k]^T @ W3[k, :, n] + ones^T @ b3[:, n]
        # (b3 enters as a rank-1 matmul so the DVE stays off the tail).
        # The ACT observer copy per W3 chunk folds that chunk's Act-ring
        # DMA completion into ACT's clock, so each matmul group needs
        # only its single ACT wait.
        for n in range(NT):
            nc.scalar.copy(
                out=obs[0:1, n + 1 : n + 2], in_=w3_sb[n][0:1, 0, 0:1]
            )
            po = pout.tile([SPC, 512], f32, tag="po")
            for k in range(K2):
                nc.tensor.matmul(
                    po[:, :],
                    h2_sb[:, k, :],
                    w3_sb[n][:, k, :],
                    start=(k == 0),
                    stop=False,
                )
            nc.tensor.matmul(
                po[:, :],
                ones[:, :],
                b3_sb[:, n * 512 : (n + 1) * 512],
                start=False,
                stop=True,
            )
            ob = outp.tile([SPC, 512], f32, tag="ob")
            nc.scalar.copy(out=ob[:, :], in_=po[:, :])
            # SWDGE store: DMASW lanes unused by the feature stream.
            nc.gpsimd.dma_start(
                out=out[:, n * 512 : (n + 1) * 512],
                in_=ob[:, :],
            )
    nc.compile()
    _build_cache[cap] = nc
    return nc


def kernel(**inputs):
    global LAST_RESULTS
    features = np.asarray(inputs["features"], dtype=np.float32)
    batch_ids = np.asarray(inputs["batch_ids"])
    W1 = np.asarray(inputs["W1"], dtype=np.float32)
    b1 = np.asarray(inputs["b1"], dtype=np.float32)
    W2 = np.asarray(inputs["W2"], dtype=np.float32)
    b2 = np.asarray(inputs["b2"], dtype=np.float32)
    W3 = np.asarray(inputs["W3"], dtype=np.float32)
    b3 = np.asarray(inputs["b3"], dtype=np.float32)

    bounds = np.searchsorted(batch_ids, np.arange(B + 1), side="left")
    seg_len = np.diff(bounds)
    maxlen = max(1, int(seg_len.max()))
    L = -(-maxlen // P)  # ceil
    L = -(-L // 4) * 4  # multiple of 4 (quarter-chunk view of segment 7)
    L = max(L, 64)  # keep L//4 >= 2*RB so the tree structure holds
    cap = L * P

    packed = np.empty((B, cap, C), np.float16)
    for b in range(B):
        lo, hi = int(bounds[b]), int(bounds[b + 1])
        n = hi - lo
        packed[b, :n] = features[lo:hi]
        packed[b, n:] = NEG

    w1h = np.ascontiguousarray(W1.astype(np.float16))
    w2h = np.ascontiguousarray(W2.astype(np.float16))
    w3h = np.ascontiguousarray(W3.astype(np.float16))
    b1t = np.ascontiguousarray(b1.reshape(K1, P).T)
    b2t = np.ascontiguousarray(b2.reshape(K2, P).T)
    b3t = np.ascontiguousarray(b3.astype(np.float16).reshape(1, OUT_D))

    nc = _build(cap)

    in_maps = []
    for d in range(NCORES):
        in_maps.append(
            {
                "feats": packed[d * SPC : (d + 1) * SPC].reshape(SPC * cap, C),
                "w1": w1h,
                "b1t": b1t,
                "w2": w2h,
                "b2t": b2t,
                "w3": w3h,
                "b3t": b3t,
            }
        )

    _ensure_axon_hooks()
    from concourse.bass_utils import run_bass_kernel_spmd

    core_ids = list(range(NCORES))
    try:
        res = run_bass_kernel_spmd(nc, in_maps, core_ids=core_ids)
    except Exception:
        if os.environ.get("BASS_TRACE") and not os.environ.get("BASS_NEVER_TRACE"):
            # trace post-processing can fail in restricted containers;
            # retry without tracing so the numeric result still lands.
            os.environ["BASS_NEVER_TRACE"] = "1"
            try:
                res = run_bass_kernel_spmd(nc, in_maps, core_ids=core_ids)
            finally:
                os.environ.pop("BASS_NEVER_TRACE", None)
        else:
            raise
    LAST_RESULTS = res

    full = np.concatenate([r["out"] for r in res.results], axis=0)
    return full.reshape(B, 3, NUM_POINTS)
